# revision 23
# baseline (speedup 1.0000x reference)
"""Trainium2 Bass kernel for nn_Attention_86655260164582 (sparse_attention).

Head-sharded tensor parallel over 8 NeuronCores: core c owns q-heads {2c, 2c+1}
and kv-head c. Each core computes its projections from full x, RoPE+RMS, the
attention (QK^T + ca_bias + alpha*conv3x3(prev_attn), causal softmax), PV, and a
partial output projection. Host sums the 8 partial out-projections (the
"all-reduce") and stacks the per-core softmax-weight shards.

Self-contained: hardcodes all shapes; no sibling imports.
"""

import sys
import numpy as np

sys.path.insert(0, "/opt/trn_rl_repo")
import ml_dtypes

import concourse.bass as bass
import concourse.mybir as mybir
import concourse.tile as tile
from concourse.bass_utils import run_bass_kernel_spmd
from concourse.masks import make_identity

F32 = mybir.dt.float32
F32R = mybir.dt.float32r
BF16 = mybir.dt.bfloat16
AF = mybir.ActivationFunctionType
ALU = mybir.AluOpType

B, T, C = 1, 2048, 1024
H, KVH, HD = 16, 8, 64
NCORE = 8
NT = T // 128  # 16 token tiles

# matmul input dtype for the big fp32 matmuls (float32r = full-rate PE).
MMDT = F32R


def r(ap):
    return ap.bitcast(MMDT)


def _split_waits(nc):
    """Workaround for this container's walrus: inline on_wait on non-
    EventSemaphore instructions fails codegen ("Too many sync wait
    commands"). Hoist each wait onto its own EventSemaphore just before
    the instruction, raw-bass style."""
    for f in nc.m.functions:
        for bb in f.blocks:
            new = []
            for inst in bb.instructions:
                si = inst.sync_info
                if si is not None and si.on_wait and not isinstance(inst, mybir.InstEventSemaphore):
                    for k, w in enumerate(si.on_wait):
                        new.append(
                            mybir.InstEventSemaphore(
                                name=f"{inst.name}-wt{k}",
                                engine=inst.engine,
                                sync_info=mybir.SyncInfo(on_wait=[w], on_update=[]),
                            )
                        )
                    si.on_wait = []
                new.append(inst)
            bb.instructions[:] = new


def build_bass():
    nc = bass.Bass()

    # ---- DRAM parameters (per-core shards; same program on all cores) ----
    xT_d = nc.declare_dram_parameter("xT", [C, T], F32R, isOutput=False)
    wqkv_d = nc.declare_dram_parameter("wqkv", [8, 128, 256], F32R, isOutput=False)
    wg_d = nc.declare_dram_parameter("wg", [12, 1], F32, isOutput=False)
    xg12_d = nc.declare_dram_parameter("xg12", [12, T], F32, isOutput=False)
    wo_d = nc.declare_dram_parameter("wo", [128, C], F32R, isOutput=False)
    ve_d = nc.declare_dram_parameter("ve3", [128, NT * 64], F32, isOutput=False)
    cos_d = nc.declare_dram_parameter("cos2", [128, NT * 64], F32, isOutput=False)
    sin_d = nc.declare_dram_parameter("sin2", [128, NT * 64], F32, isOutput=False)
    prev_d = nc.declare_dram_parameter("prev2", [2, T, T + 2], F32, isOutput=False)
    cab_d = nc.declare_dram_parameter("cab", [T, T], F32, isOutput=False)
    t6_d = nc.declare_dram_parameter("t6", [128, 6 * 128], BF16, isOutput=False)
    c16_d = nc.declare_dram_parameter("c16", [32, 16 * 128], BF16, isOutput=False)
    taps_d = nc.declare_dram_parameter("taps", [32, 6], F32, isOutput=False)
    mask_d = nc.declare_dram_parameter("mask", [128, 128], F32, isOutput=False)
    identr_d = nc.declare_dram_parameter("identb", [128, 128], BF16, isOutput=False)
    hbx_d = nc.declare_dram_parameter("hbx", [2, 32, T + 4], F32, isOutput=False)

    w_out = nc.declare_dram_parameter("w_out", [2, T, T], F32, isOutput=True)
    out_p = nc.declare_dram_parameter("out_p", [T, C], F32, isOutput=True)

    with tile.TileContext(nc) as tc:
        with (
            tc.tile_pool(name="const", bufs=1) as cpool,
            tc.tile_pool(name="persist", bufs=1) as ppool,
            tc.tile_pool(name="work", bufs=2) as wpool,
            tc.tile_pool(name="xt", bufs=4) as xpool,
            tc.tile_pool(name="small", bufs=4) as spool,
            tc.tile_pool(name="tp", bufs=2, space="PSUM") as tp_pool,
            tc.tile_pool(name="yt", bufs=2, space="PSUM") as yt_pool,
            tc.tile_pool(name="s", bufs=1, space="PSUM") as s_pool,
        ):
            # ---------- constants ----------
            ident = cpool.tile([128, 128], F32)
            make_identity(nc, ident[:])
            identb = cpool.tile([128, 128], BF16)
            nc.sync.dma_start(identb[:], identr_d[:])
            mask_sb = cpool.tile([128, 128], F32)
            nc.sync.dma_start(mask_sb[:], mask_d[:])
            wqkv_sb = cpool.tile([128, 8 * 256], F32R)
            nc.sync.dma_start(
                wqkv_sb[:].rearrange("p (c n) -> p c n", c=8), wqkv_d[:, :, :].rearrange("c p n -> p c n")
            )
            wg_sb = cpool.tile([12, 1], F32)
            nc.sync.dma_start(wg_sb[:], wg_d[:])
            wo_sb = cpool.tile([128, C], F32R)
            nc.sync.dma_start(wo_sb[:], wo_d[:])
            ve_sb = cpool.tile([128, NT * 64], F32)
            nc.sync.dma_start(ve_sb[:], ve_d[:])
            cos_sb = cpool.tile([128, NT * 64], F32)
            nc.sync.dma_start(cos_sb[:], cos_d[:])
            sin_sb = cpool.tile([128, NT * 64], F32)
            nc.sync.dma_start(sin_sb[:], sin_d[:])
            t6_sb = cpool.tile([128, 6 * 128], BF16)
            nc.sync.dma_start(t6_sb[:], t6_d[:])
            c16_sb = cpool.tile([32, 16 * 128], BF16)
            nc.sync.dma_start(c16_sb[:], c16_d[:])
            taps_sb = cpool.tile([32, 6], F32)
            nc.sync.dma_start(taps_sb[:], taps_d[:])

            # halo rows + their 1D conv, per head
            HB = []
            Hp = []
            for h in range(2):
                hb = cpool.tile([32, T + 4], F32, name=f"hb{h}")
                nc.sync.dma_start(hb[:], hbx_d[h])
                hp = cpool.tile([32, T], F32, name=f"hp{h}")
                nc.vector.tensor_scalar(hp[:], hb[:, 0:T], taps_sb[:, 3 * h : 3 * h + 1], None, op0=ALU.mult)
                nc.vector.scalar_tensor_tensor(
                    hp[:], hb[:, 1 : T + 1], taps_sb[:, 3 * h + 1 : 3 * h + 2], hp[:], op0=ALU.mult, op1=ALU.add
                )
                nc.vector.scalar_tensor_tensor(
                    hp[:], hb[:, 2 : T + 2], taps_sb[:, 3 * h + 2 : 3 * h + 3], hp[:], op0=ALU.mult, op1=ALU.add
                )
                hpr = cpool.tile([32, T], BF16, name=f"hpr{h}")
                nc.scalar.copy(hpr[:], hp[:])
                HB.append(hb)
                Hp.append(hpr)

            # persistent activations
            QT = ppool.tile([64, 2 * T], F32R)  # head-major: [:, h*T + tok]
            KT = ppool.tile([64, T], F32R)
            Vb = ppool.tile([128, NT * 64], BF16)  # 16 tiles [tok128, d64]
            yT = ppool.tile([128, T], F32R)

            # ---------- phase B: projections + rope + rms ----------
            if True:
                for i in range(NT):
                    ts = bass.ts(i, 128)
                    qkv = s_pool.tile([128, 256], F32, tag="s")
                    for cc in range(8):
                        xt = xpool.tile([128, 128], F32R, tag="xt")
                        nc.sync.dma_start(xt[:], xT_d[bass.ts(cc, 128), ts])
                        nc.tensor.matmul(
                            qkv[:],
                            xt[:],
                            wqkv_sb[:, bass.ts(cc, 256)],
                            start=(cc == 0),
                            stop=(cc == 7),
                        )
                    # gate
                    xg = spool.tile([12, 128], F32, tag="xg")
                    nc.sync.dma_start(xg[:], xg12_d[:, ts])
                    gp = yt_pool.tile([128, 1], F32, tag="ytp")
                    nc.tensor.matmul(gp[:], xg[:], wg_sb[:], start=True, stop=True)
                    gsb = spool.tile([128, 1], F32, tag="gsb")
                    nc.scalar.activation(gsb[:], gp[:], AF.Sigmoid)

                    # V = gate * (3*ve) + v   -> bf16
                    nc.vector.scalar_tensor_tensor(
                        Vb[:, bass.ts(i, 64)],
                        ve_sb[:, bass.ts(i, 64)],
                        gsb[:],
                        qkv[:, 192:256],
                        op0=ALU.mult,
                        op1=ALU.add,
                    )

                    # rope: qn/kn tiles
                    qn = wpool.tile([128, 128], F32, tag="qn")
                    kn = wpool.tile([128, 64], F32, tag="kn")
                    cosv = cos_sb[:, bass.ts(i, 64)].rearrange("p (h d) -> p h d", h=2)
                    sinv = sin_sb[:, bass.ts(i, 64)].rearrange("p (h d) -> p h d", h=2)
                    q2 = qkv[:, 0:128].rearrange("p (h d) -> p h d", h=2)
                    qo = qn[:].rearrange("p (h d) -> p h d", h=2)
                    tmp = wpool.tile([128, 64], F32, tag="tmp")
                    tmp2 = tmp[:].rearrange("p (h d) -> p h d", h=2)
                    # q half1 = x1*cos + x2*sin
                    nc.vector.tensor_tensor(qo[:, :, 0:32], q2[:, :, 0:32], cosv, op=ALU.mult)
                    nc.vector.tensor_tensor(tmp2, q2[:, :, 32:64], sinv, op=ALU.mult)
                    nc.vector.tensor_tensor(qo[:, :, 0:32], qo[:, :, 0:32], tmp2, op=ALU.add)
                    # q half2 = -x1*sin + x2*cos
                    nc.vector.tensor_tensor(qo[:, :, 32:64], q2[:, :, 32:64], cosv, op=ALU.mult)
                    nc.vector.tensor_tensor(tmp2, q2[:, :, 0:32], sinv, op=ALU.mult)
                    nc.vector.tensor_tensor(qo[:, :, 32:64], qo[:, :, 32:64], tmp2, op=ALU.subtract)
                    # k (one head): cols 128:192
                    cv1 = cos_sb[:, bass.ts(i, 64)][:, 0:32]
                    sv1 = sin_sb[:, bass.ts(i, 64)][:, 0:32]
                    tk = tmp[:, 0:32]
                    nc.vector.tensor_tensor(kn[:, 0:32], qkv[:, 128:160], cv1, op=ALU.mult)
                    nc.vector.tensor_tensor(tk, qkv[:, 160:192], sv1, op=ALU.mult)
                    nc.vector.tensor_tensor(kn[:, 0:32], kn[:, 0:32], tk, op=ALU.add)
                    nc.vector.tensor_tensor(kn[:, 32:64], qkv[:, 160:192], cv1, op=ALU.mult)
                    nc.vector.tensor_tensor(tk, qkv[:, 128:160], sv1, op=ALU.mult)
                    nc.vector.tensor_tensor(kn[:, 32:64], kn[:, 32:64], tk, op=ALU.subtract)

                    # rms per head: ss = 1e-6 + sum(q^2)/64 ; scale = c / sqrt(ss)
                    sq = wpool.tile([128, 64], F32, tag="sq")
                    ssum = spool.tile([128, 3], F32, tag="ssum")
                    rsc = spool.tile([128, 3], F32, tag="rsc")
                    for h in range(2):
                        nc.scalar.activation(
                            sq[:], qn[:, bass.ts(h, 64)], AF.Square,
                            accum_out=ssum[:, h : h + 1],
                        )
                    nc.scalar.activation(sq[:], kn[:], AF.Square, accum_out=ssum[:, 2:3])
                    nc.vector.tensor_scalar(ssum[:], ssum[:], 1.0 / 64, 1e-6, op0=ALU.mult, op1=ALU.add)
                    nc.scalar.activation(rsc[:], ssum[:], AF.Sqrt)
                    nc.vector.reciprocal(rsc[:], rsc[:])
                    # fold output scales: q: 1.2/8, k: 1.2
                    nc.vector.tensor_scalar(rsc[:, 0:2], rsc[:, 0:2], 0.15, None, op0=ALU.mult)
                    nc.vector.tensor_scalar(rsc[:, 2:3], rsc[:, 2:3], 1.2, None, op0=ALU.mult)
                    for h in range(2):
                        nc.vector.tensor_scalar(
                            qn[:, bass.ts(h, 64)], qn[:, bass.ts(h, 64)], rsc[:, h : h + 1], None, op0=ALU.mult
                        )
                    nc.vector.tensor_scalar(kn[:], kn[:], rsc[:, 2:3], None, op0=ALU.mult)

                    # transpose into QT / KT (per head so base partition stays 0)
                    for h in range(2):
                        ptq = tp_pool.tile([64, 128], F32, tag="tp")
                        nc.tensor.transpose(ptq[:], qn[:, bass.ts(h, 64)], ident[:])
                        nc.scalar.copy(QT[:, h * T + i * 128 : h * T + (i + 1) * 128], ptq[:])
                    ptk = tp_pool.tile([64, 128], F32, tag="tp")
                    nc.tensor.transpose(ptk[:], kn[:], ident[:])
                    nc.scalar.copy(KT[:, ts], ptk[:])

            # ---------- phase C: attention ----------
            if True:
                for i in range(NT):
                    Ni = (i + 1) * 128
                    nb = (Ni + 511) // 512
                    ts = bass.ts(i, 128)
                    cabt = wpool.tile([128, 2048], BF16, tag="cab")
                    nc.gpsimd.dma_start(cabt[:, 0:Ni], cab_d[ts, 0:Ni])
                    ytp = yt_pool.tile([128, 128], F32, tag="ytp")
                    for h in range(2):
                        P = wpool.tile([128, 2052], BF16, tag="P")
                        nc.gpsimd.dma_start(P[:, 0 : Ni + 2], prev_d[h, ts, 0 : Ni + 2])
                        s = s_pool.tile([128, 2048], F32, tag="s")
                        for b in range(nb):
                            cs = b * 512
                            ce = min(Ni, cs + 512)
                            nc.tensor.matmul(
                                s[:, cs:ce],
                                QT[:, h * T + i * 128 : h * T + (i + 1) * 128],
                                KT[:, cs:ce],
                                start=True, stop=False,
                            )
                            for dxi in range(3):
                                nc.tensor.matmul(
                                    s[:, cs:ce],
                                    t6_sb[:, bass.ts(3 * h + dxi, 128)],
                                    P[:, dxi + cs : dxi + ce],
                                    start=False, stop=False,
                                )
                            nc.tensor.matmul(
                                s[:, cs:ce],
                                c16_sb[:, bass.ts(i, 128)],
                                Hp[h][:, cs:ce],
                                start=False, stop=False,
                            )
                            nc.tensor.matmul(
                                s[:, cs:ce],
                                identb[:],
                                cabt[:, cs:ce],
                                start=False, stop=True,
                            )
                        # causal mask on diagonal block
                        nc.vector.tensor_tensor(s[:, i * 128 : Ni], s[:, i * 128 : Ni], mask_sb[:], op=ALU.add)
                        # softmax (no max subtraction; range is bounded)
                        wexp = wpool.tile([128, 2048], F32, tag="wexp")
                        rsum = spool.tile([128, 1], F32, tag="rsum")
                        nc.scalar.activation(wexp[:, 0:Ni], s[:, 0:Ni], AF.Exp, accum_out=rsum[:])
                        rinv = spool.tile([128, 1], F32, tag="rinv")
                        nc.vector.reciprocal(rinv[:], rsum[:])
                        wfin = wpool.tile([128, 2048], F32, tag="wfin")
                        nc.vector.tensor_scalar(wfin[:, 0:Ni], wexp[:, 0:Ni], rinv[:], None, op0=ALU.mult)
                        nc.sync.dma_start(w_out[h, ts, 0:Ni], wfin[:, 0:Ni])
                        # cast w to bf16, DMA-transpose each 128-chunk, PV
                        wcast = wpool.tile([128, 2048], BF16, tag="wcast")
                        nc.scalar.copy(wcast[:, 0:Ni], wfin[:, 0:Ni])
                        for j in range(i + 1):
                            wts = spool.tile([128, 128], BF16, tag="wts")
                            nc.sync.dma_start(wts[:], wcast[:, bass.ts(j, 128)], transpose=True)
                            nc.tensor.matmul(
                                ytp[bass.ts(h, 64), :],
                                Vb[:, bass.ts(j, 64)],
                                wts[:],
                                start=(j == 0), stop=(j == i),
                            )
                    nc.scalar.copy(yT[:, ts], ytp[:])

            # ---------- phase D: output projection ----------
            if True:
                for i in range(NT):
                    ts = bass.ts(i, 128)
                    op = s_pool.tile([128, 1024], F32, tag="s")
                    nc.tensor.matmul(op[:, 0:512], yT[:, ts], wo_sb[:, 0:512], start=True, stop=True)
                    nc.tensor.matmul(op[:, 512:1024], yT[:, ts], wo_sb[:, 512:1024], start=True, stop=True)
                    osb = wpool.tile([128, 1024], F32, tag="osb")
                    nc.vector.tensor_copy(osb[:], op[:])
                    nc.sync.dma_start(out_p[ts, :], osb[:])

    _split_waits(nc)
    return nc


_NC_CACHE = None


def _get_nc():
    global _NC_CACHE
    if _NC_CACHE is None:
        _NC_CACHE = build_bass()
    return _NC_CACHE


def _host_prep(inputs):
    """Build per-core input maps."""
    x = np.ascontiguousarray(np.asarray(inputs["x"], np.float32)[0])        # [T, C]
    ve = np.asarray(inputs["ve"], np.float32)[0]                            # [T, 512]
    cos = np.asarray(inputs["cos"], np.float32)[0, :, 0]                    # [T, 32]
    sin = np.asarray(inputs["sin"], np.float32)[0, :, 0]
    prev = np.asarray(inputs["prev_attn"], np.float32)[0]                   # [16, T, T]
    Wq = np.asarray(inputs["Wq"], np.float32)
    Wk = np.asarray(inputs["Wk"], np.float32)
    Wv = np.asarray(inputs["Wv"], np.float32)
    Wo = np.asarray(inputs["Wo"], np.float32)
    Wg = np.asarray(inputs["Wg"], np.float32)
    cab = np.ascontiguousarray(np.asarray(inputs["ca_bias"], np.float32)[0, 0])
    conv_w = np.asarray(inputs["conv_w"], np.float32)
    alpha = float(np.asarray(inputs["alpha"]))

    xT = np.ascontiguousarray(x.T)                                          # [C, T]
    cos2 = np.concatenate([cos, cos], axis=1)                               # [T, 64]
    sin2 = np.concatenate([sin, sin], axis=1)
    # [T,64] -> [128, NT*64] tile-major layout
    def tiles128(a):
        return np.ascontiguousarray(
            a.reshape(NT, 128, a.shape[1]).transpose(1, 0, 2).reshape(128, -1)
        )

    cos2t = tiles128(cos2)
    sin2t = tiles128(sin2)

    mask = np.triu(np.full((128, 128), -1e30, np.float32), 1)
    C16 = np.zeros((16, 32, 128), np.float32)
    for i in range(16):
        top = 0 if i == 0 else 2 * i - 1
        bot = 2 * i + 2 if i < 15 else 31
        C16[i, top, 0] = 1.0
        C16[i, bot, 127] = 1.0
    c16_flat = np.ascontiguousarray(C16.transpose(1, 0, 2).reshape(32, 16 * 128))

    in_maps = []
    for c in range(NCORE):
        h0 = 2 * c
        wq_s = Wq[:, h0 * 64 : (h0 + 2) * 64]                               # [1024, 128]
        wkv = np.concatenate([Wk[:, c * 64 : (c + 1) * 64], Wv[:, c * 64 : (c + 1) * 64]], axis=1)
        wqkv = np.concatenate([wq_s, wkv], axis=1).reshape(8, 128, 256)     # [8, 128, 256]
        cw2 = conv_w[h0 : h0 + 2, 0] * alpha                                # [2,3,3]
        T6 = np.zeros((6, 128, 128), np.float32)
        for h in range(2):
            for dxi in range(3):
                for dy in (-1, 0, 1):
                    val = cw2[h, dy + 1, dxi]
                    idx = np.arange(max(0, -dy), min(128, 128 - dy))
                    T6[h * 3 + dxi, idx + dy, idx] = val
        t6_flat = np.ascontiguousarray(T6.transpose(1, 0, 2).reshape(128, 6 * 128))
        taps = np.zeros((32, 6), np.float32)
        for h in range(2):
            jr = np.arange(1, 16)
            taps[2 * jr - 1, 3 * h : 3 * h + 3] = cw2[h, 0, :]
            taps[2 * jr, 3 * h : 3 * h + 3] = cw2[h, 2, :]
        prev2p = np.zeros((2, T, T + 2), np.float32)
        prev2p[:, :, 1 : T + 1] = prev[h0 : h0 + 2]
        hbx = np.zeros((2, 32, T + 4), np.float32)
        for h in range(2):
            jr = np.arange(1, 16)
            hbx[h, 2 * jr - 1, 1 : T + 1] = prev[h0 + h, 128 * jr - 1, :]
            hbx[h, 2 * jr, 1 : T + 1] = prev[h0 + h, 128 * jr, :]
        in_maps.append(
            {
                "xT": xT,
                "wqkv": np.ascontiguousarray(wqkv),
                "wg": np.ascontiguousarray(Wg[:, c : c + 1]),
                "wo": np.ascontiguousarray(Wo[128 * c : 128 * (c + 1), :]),
                "ve3": tiles128(3.0 * ve[:, c * 64 : (c + 1) * 64]),
                "cos2": cos2t,
                "sin2": sin2t,
                "prev2": prev2p,
                "cab": cab,
                "t6": t6_flat.astype(ml_dtypes.bfloat16),
                "c16": c16_flat.astype(ml_dtypes.bfloat16),
                "taps": taps,
                "mask": mask,
                "identb": np.eye(128, dtype=np.float32).astype(ml_dtypes.bfloat16),
                "xg12": np.ascontiguousarray(xT[0:12, :]),
                "hbx": hbx,
            }
        )
    return in_maps


def kernel(**inputs):
    nc = _get_nc()
    in_maps = _host_prep(inputs)
    res = run_bass_kernel_spmd(nc, in_maps, core_ids=list(range(NCORE)))
    outs = res.results
    out = np.sum([o["out_p"] for o in outs], axis=0)[None]                  # [1, T, C]
    w = np.concatenate([o["w_out"] for o in outs], axis=0)[None]            # [1, 16, T, T]
    return out.astype(np.float32), w.astype(np.float32)


if __name__ == "__main__":
    nc = build_bass()
    print("built ok")


# revision 25
# speedup vs baseline: 1.5531x; 1.5531x over previous
"""Trainium2 Bass kernel for nn_Attention_86655260164582 (sparse_attention).

Head-sharded tensor parallel over 8 NeuronCores: core c owns q-heads {2c, 2c+1}
and kv-head c. Each core computes its projections from full x, RoPE+RMS, the
attention (QK^T + ca_bias + alpha*conv3x3(prev_attn), causal softmax), PV, and a
partial output projection. Host sums the 8 partial out-projections (the
"all-reduce") and stacks the per-core softmax-weight shards.

Self-contained: hardcodes all shapes; no sibling imports.
"""

import sys
import numpy as np

sys.path.insert(0, "/opt/trn_rl_repo")
import ml_dtypes

import concourse.bass as bass
import concourse.mybir as mybir
import concourse.tile as tile
from concourse.bass_utils import run_bass_kernel_spmd
from concourse.masks import make_identity

F32 = mybir.dt.float32
F32R = mybir.dt.float32r
BF16 = mybir.dt.bfloat16
AF = mybir.ActivationFunctionType
ALU = mybir.AluOpType

B, T, C = 1, 2048, 1024
H, KVH, HD = 16, 8, 64
NCORE = 8
NT = T // 128  # 16 token tiles

# matmul input dtype for the big fp32 matmuls (float32r = full-rate PE).
MMDT = F32R


def r(ap):
    return ap.bitcast(MMDT)


def _split_waits(nc):
    """Workaround for this container's walrus: inline on_wait on non-
    EventSemaphore instructions fails codegen ("Too many sync wait
    commands"). Hoist each wait onto its own EventSemaphore just before
    the instruction, raw-bass style."""
    for f in nc.m.functions:
        for bb in f.blocks:
            new = []
            for inst in bb.instructions:
                si = inst.sync_info
                if si is not None and si.on_wait and not isinstance(inst, mybir.InstEventSemaphore):
                    for k, w in enumerate(si.on_wait):
                        new.append(
                            mybir.InstEventSemaphore(
                                name=f"{inst.name}-wt{k}",
                                engine=inst.engine,
                                sync_info=mybir.SyncInfo(on_wait=[w], on_update=[]),
                            )
                        )
                    si.on_wait = []
                new.append(inst)
            bb.instructions[:] = new


def build_bass():
    nc = bass.Bass()

    # ---- DRAM parameters (per-core shards; same program on all cores) ----
    xT_d = nc.declare_dram_parameter("xT", [C, T], F32R, isOutput=False)
    wqkv_d = nc.declare_dram_parameter("wqkv", [8, 128, 256], F32R, isOutput=False)
    wg_d = nc.declare_dram_parameter("wg", [12, 1], F32, isOutput=False)
    xg12_d = nc.declare_dram_parameter("xg12", [12, T], F32, isOutput=False)
    wo_d = nc.declare_dram_parameter("wo", [128, C], BF16, isOutput=False)
    ve_d = nc.declare_dram_parameter("ve3", [128, NT * 64], F32, isOutput=False)
    cos_d = nc.declare_dram_parameter("cos2", [128, NT * 64], F32, isOutput=False)
    sin_d = nc.declare_dram_parameter("sin2", [128, NT * 64], F32, isOutput=False)
    prev_d = nc.declare_dram_parameter("prev2", [2, T, T + 2], F32, isOutput=False)
    cab_d = nc.declare_dram_parameter("cab", [T, T], F32, isOutput=False)
    t6_d = nc.declare_dram_parameter("t6", [128, 6 * 128], BF16, isOutput=False)
    c16_d = nc.declare_dram_parameter("c16", [32, 16 * 128], BF16, isOutput=False)
    taps_d = nc.declare_dram_parameter("taps", [32, 6], F32, isOutput=False)
    mask_d = nc.declare_dram_parameter("mask", [128, 128], F32, isOutput=False)
    identr_d = nc.declare_dram_parameter("identb", [128, 128], BF16, isOutput=False)
    hbx_d = nc.declare_dram_parameter("hbx", [2, 32, T + 4], F32, isOutput=False)

    w_out = nc.declare_dram_parameter("w_out", [2, T, T], F32, isOutput=True)
    out_p = nc.declare_dram_parameter("out_p", [T, C], F32, isOutput=True)

    with tile.TileContext(nc) as tc:
        with (
            tc.tile_pool(name="const", bufs=1) as cpool,
            tc.tile_pool(name="persist", bufs=1) as ppool,
            tc.tile_pool(name="work", bufs=2) as wpool,
            tc.tile_pool(name="xt", bufs=4) as xpool,
            tc.tile_pool(name="small", bufs=4) as spool,
            tc.tile_pool(name="tp", bufs=2, space="PSUM") as tp_pool,
            tc.tile_pool(name="yt", bufs=2, space="PSUM") as yt_pool,
            tc.tile_pool(name="s", bufs=1, space="PSUM") as s_pool,
        ):
            # ---------- constants ----------
            ident = cpool.tile([128, 128], F32)
            make_identity(nc, ident[:])
            identb = cpool.tile([128, 128], BF16)
            nc.sync.dma_start(identb[:], identr_d[:])
            mask_sb = cpool.tile([128, 128], F32)
            nc.sync.dma_start(mask_sb[:], mask_d[:])
            wqkv_sb = cpool.tile([128, 8 * 256], F32R)
            nc.sync.dma_start(
                wqkv_sb[:].rearrange("p (c n) -> p c n", c=8), wqkv_d[:, :, :].rearrange("c p n -> p c n")
            )
            wg_sb = cpool.tile([12, 1], F32)
            nc.sync.dma_start(wg_sb[:], wg_d[:])
            wo_sb = cpool.tile([128, C], BF16)
            nc.sync.dma_start(wo_sb[:], wo_d[:])
            ve_sb = cpool.tile([128, NT * 64], F32)
            nc.sync.dma_start(ve_sb[:], ve_d[:])
            cos_sb = cpool.tile([128, NT * 64], F32)
            nc.sync.dma_start(cos_sb[:], cos_d[:])
            sin_sb = cpool.tile([128, NT * 64], F32)
            nc.sync.dma_start(sin_sb[:], sin_d[:])
            t6_sb = cpool.tile([128, 6 * 128], BF16)
            nc.sync.dma_start(t6_sb[:], t6_d[:])
            c16_sb = cpool.tile([32, 16 * 128], BF16)
            nc.sync.dma_start(c16_sb[:], c16_d[:])
            taps_sb = cpool.tile([32, 6], F32)
            nc.sync.dma_start(taps_sb[:], taps_d[:])

            # halo rows + their 1D conv, per head
            HB = []
            Hp = []
            for h in range(2):
                hb = cpool.tile([32, T + 4], F32, name=f"hb{h}")
                nc.sync.dma_start(hb[:], hbx_d[h])
                hp = cpool.tile([32, T], F32, name=f"hp{h}")
                nc.vector.tensor_scalar(hp[:], hb[:, 0:T], taps_sb[:, 3 * h : 3 * h + 1], None, op0=ALU.mult)
                nc.vector.scalar_tensor_tensor(
                    hp[:], hb[:, 1 : T + 1], taps_sb[:, 3 * h + 1 : 3 * h + 2], hp[:], op0=ALU.mult, op1=ALU.add
                )
                nc.vector.scalar_tensor_tensor(
                    hp[:], hb[:, 2 : T + 2], taps_sb[:, 3 * h + 2 : 3 * h + 3], hp[:], op0=ALU.mult, op1=ALU.add
                )
                hpr = cpool.tile([32, T], BF16, name=f"hpr{h}")
                nc.scalar.copy(hpr[:], hp[:])
                HB.append(hb)
                Hp.append(hpr)

            # persistent activations
            QT = ppool.tile([64, 2 * T], BF16)  # head-major: [:, h*T + tok]
            KT = ppool.tile([64, T], BF16)
            Vb = ppool.tile([128, NT * 64], BF16)  # 16 tiles [tok128, d64]
            yT = ppool.tile([128, T], BF16)

            # ---------- phase B: projections + rope + rms ----------
            if True:
                for i in range(NT):
                    ts = bass.ts(i, 128)
                    qkv = s_pool.tile([128, 256], F32, tag="s")
                    for cc in range(8):
                        xt = xpool.tile([128, 128], F32R, tag="xt")
                        nc.sync.dma_start(xt[:], xT_d[bass.ts(cc, 128), ts])
                        nc.tensor.matmul(
                            qkv[:],
                            xt[:],
                            wqkv_sb[:, bass.ts(cc, 256)],
                            start=(cc == 0),
                            stop=(cc == 7),
                        )
                    # gate
                    xg = spool.tile([12, 128], F32, tag="xg")
                    nc.sync.dma_start(xg[:], xg12_d[:, ts])
                    gp = yt_pool.tile([128, 1], F32, tag="ytp")
                    nc.tensor.matmul(gp[:], xg[:], wg_sb[:], start=True, stop=True)
                    gsb = spool.tile([128, 1], F32, tag="gsb")
                    nc.scalar.activation(gsb[:], gp[:], AF.Sigmoid)

                    # V = gate * (3*ve) + v   -> bf16
                    nc.vector.scalar_tensor_tensor(
                        Vb[:, bass.ts(i, 64)],
                        ve_sb[:, bass.ts(i, 64)],
                        gsb[:],
                        qkv[:, 192:256],
                        op0=ALU.mult,
                        op1=ALU.add,
                    )

                    # rope: qn/kn tiles
                    qn = wpool.tile([128, 128], F32, tag="qn")
                    kn = wpool.tile([128, 64], F32, tag="kn")
                    cosv = cos_sb[:, bass.ts(i, 64)].rearrange("p (h d) -> p h d", h=2)
                    sinv = sin_sb[:, bass.ts(i, 64)].rearrange("p (h d) -> p h d", h=2)
                    q2 = qkv[:, 0:128].rearrange("p (h d) -> p h d", h=2)
                    qo = qn[:].rearrange("p (h d) -> p h d", h=2)
                    tmp = wpool.tile([128, 64], F32, tag="tmp")
                    tmp2 = tmp[:].rearrange("p (h d) -> p h d", h=2)
                    # q half1 = x1*cos + x2*sin
                    nc.vector.tensor_tensor(qo[:, :, 0:32], q2[:, :, 0:32], cosv, op=ALU.mult)
                    nc.vector.tensor_tensor(tmp2, q2[:, :, 32:64], sinv, op=ALU.mult)
                    nc.vector.tensor_tensor(qo[:, :, 0:32], qo[:, :, 0:32], tmp2, op=ALU.add)
                    # q half2 = -x1*sin + x2*cos
                    nc.vector.tensor_tensor(qo[:, :, 32:64], q2[:, :, 32:64], cosv, op=ALU.mult)
                    nc.vector.tensor_tensor(tmp2, q2[:, :, 0:32], sinv, op=ALU.mult)
                    nc.vector.tensor_tensor(qo[:, :, 32:64], qo[:, :, 32:64], tmp2, op=ALU.subtract)
                    # k (one head): cols 128:192
                    cv1 = cos_sb[:, bass.ts(i, 64)][:, 0:32]
                    sv1 = sin_sb[:, bass.ts(i, 64)][:, 0:32]
                    tk = tmp[:, 0:32]
                    nc.vector.tensor_tensor(kn[:, 0:32], qkv[:, 128:160], cv1, op=ALU.mult)
                    nc.vector.tensor_tensor(tk, qkv[:, 160:192], sv1, op=ALU.mult)
                    nc.vector.tensor_tensor(kn[:, 0:32], kn[:, 0:32], tk, op=ALU.add)
                    nc.vector.tensor_tensor(kn[:, 32:64], qkv[:, 160:192], cv1, op=ALU.mult)
                    nc.vector.tensor_tensor(tk, qkv[:, 128:160], sv1, op=ALU.mult)
                    nc.vector.tensor_tensor(kn[:, 32:64], kn[:, 32:64], tk, op=ALU.subtract)

                    # rms per head: ss = 1e-6 + sum(q^2)/64 ; scale = c / sqrt(ss)
                    sq = wpool.tile([128, 64], F32, tag="sq")
                    ssum = spool.tile([128, 3], F32, tag="ssum")
                    rsc = spool.tile([128, 3], F32, tag="rsc")
                    for h in range(2):
                        nc.scalar.activation(
                            sq[:], qn[:, bass.ts(h, 64)], AF.Square,
                            accum_out=ssum[:, h : h + 1],
                        )
                    nc.scalar.activation(sq[:], kn[:], AF.Square, accum_out=ssum[:, 2:3])
                    nc.vector.tensor_scalar(ssum[:], ssum[:], 1.0 / 64, 1e-6, op0=ALU.mult, op1=ALU.add)
                    nc.scalar.activation(rsc[:], ssum[:], AF.Sqrt)
                    nc.vector.reciprocal(rsc[:], rsc[:])
                    # fold output scales: q: 1.2/8, k: 1.2
                    nc.vector.tensor_scalar(rsc[:, 0:2], rsc[:, 0:2], 0.15, None, op0=ALU.mult)
                    nc.vector.tensor_scalar(rsc[:, 2:3], rsc[:, 2:3], 1.2, None, op0=ALU.mult)
                    for h in range(2):
                        nc.vector.tensor_scalar(
                            qn[:, bass.ts(h, 64)], qn[:, bass.ts(h, 64)], rsc[:, h : h + 1], None, op0=ALU.mult
                        )
                    nc.vector.tensor_scalar(kn[:], kn[:], rsc[:, 2:3], None, op0=ALU.mult)

                    # transpose into QT / KT (per head so base partition stays 0)
                    for h in range(2):
                        ptq = tp_pool.tile([64, 128], F32, tag="tp")
                        nc.tensor.transpose(ptq[:], qn[:, bass.ts(h, 64)], ident[:])
                        nc.scalar.copy(QT[:, h * T + i * 128 : h * T + (i + 1) * 128], ptq[:])
                    ptk = tp_pool.tile([64, 128], F32, tag="tp")
                    nc.tensor.transpose(ptk[:], kn[:], ident[:])
                    nc.scalar.copy(KT[:, ts], ptk[:])

            # ---------- phase C: attention ----------
            if True:
                for i in range(NT):
                    Ni = (i + 1) * 128
                    nb = (Ni + 511) // 512
                    ts = bass.ts(i, 128)
                    cabt = wpool.tile([128, 2048], BF16, tag="cab")
                    nc.gpsimd.dma_start(cabt[:, 0:Ni], cab_d[ts, 0:Ni])
                    ytp = yt_pool.tile([128, 128], F32, tag="ytp")
                    for h in range(2):
                        P = wpool.tile([128, 2052], BF16, tag="P")
                        nc.gpsimd.dma_start(P[:, 0 : Ni + 2], prev_d[h, ts, 0 : Ni + 2])
                        s = s_pool.tile([128, 2048], F32, tag="s")
                        for b in range(nb):
                            cs = b * 512
                            ce = min(Ni, cs + 512)
                            nc.tensor.matmul(
                                s[:, cs:ce],
                                QT[:, h * T + i * 128 : h * T + (i + 1) * 128],
                                KT[:, cs:ce],
                                start=True, stop=False,
                            )
                            for dxi in range(3):
                                nc.tensor.matmul(
                                    s[:, cs:ce],
                                    t6_sb[:, bass.ts(3 * h + dxi, 128)],
                                    P[:, dxi + cs : dxi + ce],
                                    start=False, stop=False,
                                )
                            nc.tensor.matmul(
                                s[:, cs:ce],
                                c16_sb[:, bass.ts(i, 128)],
                                Hp[h][:, cs:ce],
                                start=False, stop=False,
                            )
                            nc.tensor.matmul(
                                s[:, cs:ce],
                                identb[:],
                                cabt[:, cs:ce],
                                start=False, stop=True,
                            )
                        # causal mask on diagonal block
                        nc.vector.tensor_tensor(s[:, i * 128 : Ni], s[:, i * 128 : Ni], mask_sb[:], op=ALU.add)
                        # softmax (no max subtraction; range is bounded)
                        wexp = wpool.tile([128, 2048], F32, tag="wexp")
                        rsum = spool.tile([128, 1], F32, tag="rsum")
                        nc.scalar.activation(wexp[:, 0:Ni], s[:, 0:Ni], AF.Exp, accum_out=rsum[:])
                        rinv = spool.tile([128, 1], F32, tag="rinv")
                        nc.vector.reciprocal(rinv[:], rsum[:])
                        wfin = wpool.tile([128, 2048], F32, tag="wfin")
                        nc.vector.tensor_scalar(wfin[:, 0:Ni], wexp[:, 0:Ni], rinv[:], None, op0=ALU.mult)
                        nc.sync.dma_start(w_out[h, ts, 0:Ni], wfin[:, 0:Ni])
                        # cast w to bf16, PE-transpose each 128-chunk, PV
                        wcast = wpool.tile([128, 2048], BF16, tag="wcast")
                        nc.scalar.copy(wcast[:, 0:Ni], wfin[:, 0:Ni])
                        for j in range(i + 1):
                            wtp = tp_pool.tile([128, 128], BF16, tag="tp")
                            nc.tensor.transpose(wtp[:], wcast[:, bass.ts(j, 128)], identb[:])
                            wts = spool.tile([128, 128], BF16, tag="wts")
                            if j % 2 == 0:
                                nc.vector.tensor_copy(wts[:], wtp[:])
                            else:
                                nc.scalar.copy(wts[:], wtp[:])
                            nc.tensor.matmul(
                                ytp[bass.ts(h, 64), :],
                                Vb[:, bass.ts(j, 64)],
                                wts[:],
                                start=(j == 0), stop=(j == i),
                            )
                    nc.scalar.copy(yT[:, ts], ytp[:])

            # ---------- phase D: output projection ----------
            if True:
                for i in range(NT):
                    ts = bass.ts(i, 128)
                    op = s_pool.tile([128, 1024], F32, tag="s")
                    nc.tensor.matmul(op[:, 0:512], yT[:, ts], wo_sb[:, 0:512], start=True, stop=True)
                    nc.tensor.matmul(op[:, 512:1024], yT[:, ts], wo_sb[:, 512:1024], start=True, stop=True)
                    osb = wpool.tile([128, 1024], F32, tag="osb")
                    nc.vector.tensor_copy(osb[:], op[:])
                    nc.sync.dma_start(out_p[ts, :], osb[:])

    _split_waits(nc)
    return nc


_NC_CACHE = None


def _get_nc():
    global _NC_CACHE
    if _NC_CACHE is None:
        _NC_CACHE = build_bass()
    return _NC_CACHE


def _host_prep(inputs):
    """Build per-core input maps."""
    x = np.ascontiguousarray(np.asarray(inputs["x"], np.float32)[0])        # [T, C]
    ve = np.asarray(inputs["ve"], np.float32)[0]                            # [T, 512]
    cos = np.asarray(inputs["cos"], np.float32)[0, :, 0]                    # [T, 32]
    sin = np.asarray(inputs["sin"], np.float32)[0, :, 0]
    prev = np.asarray(inputs["prev_attn"], np.float32)[0]                   # [16, T, T]
    Wq = np.asarray(inputs["Wq"], np.float32)
    Wk = np.asarray(inputs["Wk"], np.float32)
    Wv = np.asarray(inputs["Wv"], np.float32)
    Wo = np.asarray(inputs["Wo"], np.float32)
    Wg = np.asarray(inputs["Wg"], np.float32)
    cab = np.ascontiguousarray(np.asarray(inputs["ca_bias"], np.float32)[0, 0])
    conv_w = np.asarray(inputs["conv_w"], np.float32)
    alpha = float(np.asarray(inputs["alpha"]))

    xT = np.ascontiguousarray(x.T)                                          # [C, T]
    cos2 = np.concatenate([cos, cos], axis=1)                               # [T, 64]
    sin2 = np.concatenate([sin, sin], axis=1)
    # [T,64] -> [128, NT*64] tile-major layout
    def tiles128(a):
        return np.ascontiguousarray(
            a.reshape(NT, 128, a.shape[1]).transpose(1, 0, 2).reshape(128, -1)
        )

    cos2t = tiles128(cos2)
    sin2t = tiles128(sin2)

    mask = np.triu(np.full((128, 128), -1e30, np.float32), 1)
    C16 = np.zeros((16, 32, 128), np.float32)
    for i in range(16):
        top = 0 if i == 0 else 2 * i - 1
        bot = 2 * i + 2 if i < 15 else 31
        C16[i, top, 0] = 1.0
        C16[i, bot, 127] = 1.0
    c16_flat = np.ascontiguousarray(C16.transpose(1, 0, 2).reshape(32, 16 * 128))

    in_maps = []
    for c in range(NCORE):
        h0 = 2 * c
        wq_s = Wq[:, h0 * 64 : (h0 + 2) * 64]                               # [1024, 128]
        wkv = np.concatenate([Wk[:, c * 64 : (c + 1) * 64], Wv[:, c * 64 : (c + 1) * 64]], axis=1)
        wqkv = np.concatenate([wq_s, wkv], axis=1).reshape(8, 128, 256)     # [8, 128, 256]
        cw2 = conv_w[h0 : h0 + 2, 0] * alpha                                # [2,3,3]
        T6 = np.zeros((6, 128, 128), np.float32)
        for h in range(2):
            for dxi in range(3):
                for dy in (-1, 0, 1):
                    val = cw2[h, dy + 1, dxi]
                    idx = np.arange(max(0, -dy), min(128, 128 - dy))
                    T6[h * 3 + dxi, idx + dy, idx] = val
        t6_flat = np.ascontiguousarray(T6.transpose(1, 0, 2).reshape(128, 6 * 128))
        taps = np.zeros((32, 6), np.float32)
        for h in range(2):
            jr = np.arange(1, 16)
            taps[2 * jr - 1, 3 * h : 3 * h + 3] = cw2[h, 0, :]
            taps[2 * jr, 3 * h : 3 * h + 3] = cw2[h, 2, :]
        prev2p = np.zeros((2, T, T + 2), np.float32)
        prev2p[:, :, 1 : T + 1] = prev[h0 : h0 + 2]
        hbx = np.zeros((2, 32, T + 4), np.float32)
        for h in range(2):
            jr = np.arange(1, 16)
            hbx[h, 2 * jr - 1, 1 : T + 1] = prev[h0 + h, 128 * jr - 1, :]
            hbx[h, 2 * jr, 1 : T + 1] = prev[h0 + h, 128 * jr, :]
        in_maps.append(
            {
                "xT": xT,
                "wqkv": np.ascontiguousarray(wqkv),
                "wg": np.ascontiguousarray(Wg[:, c : c + 1]),
                "wo": np.ascontiguousarray(Wo[128 * c : 128 * (c + 1), :]).astype(ml_dtypes.bfloat16),
                "ve3": tiles128(3.0 * ve[:, c * 64 : (c + 1) * 64]),
                "cos2": cos2t,
                "sin2": sin2t,
                "prev2": prev2p,
                "cab": cab,
                "t6": t6_flat.astype(ml_dtypes.bfloat16),
                "c16": c16_flat.astype(ml_dtypes.bfloat16),
                "taps": taps,
                "mask": mask,
                "identb": np.eye(128, dtype=np.float32).astype(ml_dtypes.bfloat16),
                "xg12": np.ascontiguousarray(xT[0:12, :]),
                "hbx": hbx,
            }
        )
    return in_maps


def kernel(**inputs):
    nc = _get_nc()
    in_maps = _host_prep(inputs)
    res = run_bass_kernel_spmd(nc, in_maps, core_ids=list(range(NCORE)))
    outs = res.results
    out = np.sum([o["out_p"] for o in outs], axis=0)[None]                  # [1, T, C]
    w = np.concatenate([o["w_out"] for o in outs], axis=0)[None]            # [1, 16, T, T]
    return out.astype(np.float32), w.astype(np.float32)


if __name__ == "__main__":
    nc = build_bass()
    print("built ok")
